# revision 8
# baseline (speedup 1.0000x reference)
"""Causal depthwise-conv MLP block (W_in -> causal conv K=4 -> SiLU -> W_out)
as a Bass/Tile kernel running data-parallel on 8 Trainium2 NeuronCores.

Sharding: (batch=4) x (sequence halves=2) -> 8 shards of 2048 sequence rows.
The causal conv needs 3 columns of left context; those are computed on the
host (exact fp32) and passed per-core, with the input-projection bias b_in
folded out of the conv input and into the SiLU bias so zero-padding at batch
starts is exact.

On-chip layout is channel-major ([C,seq] on partitions) so the depthwise conv
is per-partition scalar multiplies along the free dim.
"""

import numpy as np
import ml_dtypes

P = 128
B, S, H, C, K = 4, 4096, 2048, 4096, 4
NCORES = 8
N = S // 2          # sequence rows per core
KH = H // P         # 16 contraction tiles for the input projection
CT = C // P         # 32 channel tiles
MT = H // P         # 16 output row tiles
SUP = 1024          # sequence super-chunk held in SBUF as Y
NSUP = N // SUP     # 2
SUB = 512           # matmul moving free dim / PSUM bank
NSUB = SUP // SUB   # 2

_NC = None
LAST_RESULT = None


def _build_nc():
    import concourse.bass as bass
    import concourse.bacc as bacc
    import concourse.mybir as mybir
    from concourse.tile import TileContext
    from contextlib import ExitStack

    fp32 = mybir.dt.float32
    bf16 = mybir.dt.bfloat16
    AF = mybir.ActivationFunctionType

    nc = bacc.Bacc()
    hsT = nc.declare_dram_parameter("hsT", [H, N], bf16, isOutput=False)
    w_in = nc.declare_dram_parameter("w_in", [CT, P, KH * P], bf16, isOutput=False)
    w_out = nc.declare_dram_parameter("w_out", [MT, P, CT * P], bf16, isOutput=False)
    convw = nc.declare_dram_parameter("convw", [P, CT * 4], fp32, isOutput=False)
    biasf = nc.declare_dram_parameter("biasf", [P, CT], fp32, isOutput=False)
    halo = nc.declare_dram_parameter("halo", [P, CT * 3], fp32, isOutput=False)
    bout = nc.declare_dram_parameter("bout", [P, MT], fp32, isOutput=False)
    outT = nc.declare_dram_parameter("outT", [H, N], fp32, isOutput=True)

    with TileContext(nc) as tc, ExitStack() as ctx:
        const = ctx.enter_context(tc.tile_pool(name="const", bufs=1))
        hs_pool = ctx.enter_context(tc.tile_pool(name="hs", bufs=2))
        wi_pool = ctx.enter_context(tc.tile_pool(name="wi", bufs=2))
        wo_pool = ctx.enter_context(tc.tile_pool(name="wo", bufs=2))
        xs_pool = ctx.enter_context(tc.tile_pool(name="xs", bufs=2))
        ya_pool = ctx.enter_context(tc.tile_pool(name="ya", bufs=2))
        tm_pool = ctx.enter_context(tc.tile_pool(name="tm", bufs=1))
        yb_pool = ctx.enter_context(tc.tile_pool(name="yb", bufs=1))
        ob_pool = ctx.enter_context(tc.tile_pool(name="ob", bufs=2))
        psA = ctx.enter_context(tc.tile_pool(name="psA", bufs=2, space="PSUM"))
        psB = ctx.enter_context(tc.tile_pool(name="psB", bufs=2, space="PSUM"))

        cw = const.tile([P, CT * 4], fp32, tag="cw")
        nc.sync.dma_start(out=cw, in_=convw[:, :])
        bf = const.tile([P, CT], fp32, tag="bf")
        nc.sync.dma_start(out=bf, in_=biasf[:, :])
        hl = const.tile([P, CT * 3], fp32, tag="hl")
        nc.sync.dma_start(out=hl, in_=halo[:, :])
        bo = const.tile([P, MT], fp32, tag="bo")
        nc.sync.dma_start(out=bo, in_=bout[:, :])
        # last 3 conv-input columns of each channel tile, carried across supers
        xtail = const.tile([P, CT * 3], fp32, tag="xtail")

        for s in range(NSUP):
            hst = hs_pool.tile([P, KH * SUP], bf16, tag="hs")
            for k in range(KH):
                nc.sync.dma_start(
                    out=hst[:, k * SUP:(k + 1) * SUP],
                    in_=hsT[k * P:(k + 1) * P, s * SUP:(s + 1) * SUP],
                )
            ybig = yb_pool.tile([P, CT * SUP], bf16, tag="yb")

            # Phase A: x = W_in @ hs (bf16 matmul, fp32 psum) -> conv -> silu -> Y
            for ci in range(CT):
                wi = wi_pool.tile([P, KH * P], bf16, tag="wi")
                nc.sync.dma_start(out=wi, in_=w_in[ci])
                xs = xs_pool.tile([P, 3 + SUP], fp32, tag="xs")
                if s == 0:
                    nc.vector.tensor_copy(xs[:, 0:3], hl[:, ci * 3:ci * 3 + 3])
                else:
                    nc.vector.tensor_copy(xs[:, 0:3], xtail[:, ci * 3:ci * 3 + 3])
                for sub in range(NSUB):
                    off = sub * SUB
                    px = psA.tile([P, SUB], fp32, tag="px")
                    for k in range(KH):
                        nc.tensor.matmul(
                            px,
                            wi[:, k * P:(k + 1) * P],
                            hst[:, k * SUP + off:k * SUP + off + SUB],
                            start=(k == 0),
                            stop=(k == KH - 1),
                        )
                    nc.scalar.copy(xs[:, 3 + off:3 + off + SUB], px)
                if s + 1 < NSUP:
                    nc.vector.tensor_copy(
                        xtail[:, ci * 3:ci * 3 + 3], xs[:, SUP:SUP + 3]
                    )
                ya = ya_pool.tile([P, SUP], fp32, tag="ya")
                nc.vector.tensor_scalar_mul(
                    ya, xs[:, 0:SUP], cw[:, ci * 4:ci * 4 + 1]
                )
                for t in range(1, 4):
                    tm = tm_pool.tile([P, SUP], fp32, tag="tm")
                    nc.vector.tensor_scalar_mul(
                        tm, xs[:, t:t + SUP], cw[:, ci * 4 + t:ci * 4 + t + 1]
                    )
                    nc.vector.tensor_add(ya, ya, tm)
                nc.scalar.activation(
                    ybig[:, ci * SUP:(ci + 1) * SUP],
                    ya,
                    AF.Silu,
                    bias=bf[:, ci:ci + 1],
                    scale=1.0,
                )

            # Phase B: out = W_out @ Y (accumulate over all channel tiles)
            for m in range(MT):
                wo = wo_pool.tile([P, CT * P], bf16, tag="wo")
                nc.sync.dma_start(out=wo, in_=w_out[m])
                for sub in range(NSUB):
                    off = sub * SUB
                    po = psB.tile([P, SUB], fp32, tag="po")
                    for ci2 in range(CT):
                        nc.tensor.matmul(
                            po,
                            wo[:, ci2 * P:(ci2 + 1) * P],
                            ybig[:, ci2 * SUP + off:ci2 * SUP + off + SUB],
                            start=(ci2 == 0),
                            stop=(ci2 == CT - 1),
                        )
                    ob = ob_pool.tile([P, SUB], fp32, tag="ob")
                    nc.scalar.activation(
                        ob, po, AF.Identity, bias=bo[:, m:m + 1], scale=1.0
                    )
                    nc.sync.dma_start(
                        out=outT[m * P:(m + 1) * P, s * SUP + off:s * SUP + off + SUB],
                        in_=ob,
                    )
    nc.finalize()
    return nc


def _prep_inputs(hidden_states, W_in, b_in, conv_w, conv_b, W_out, b_out):
    bf16 = ml_dtypes.bfloat16
    f32 = np.float32
    hidden_states = np.asarray(hidden_states, f32)
    W_in = np.asarray(W_in, f32)
    b_in = np.asarray(b_in, f32)
    conv_w = np.asarray(conv_w, f32)
    conv_b = np.asarray(conv_b, f32)
    W_out = np.asarray(W_out, f32)
    b_out = np.asarray(b_out, f32)

    w_in2 = np.ascontiguousarray(
        W_in.reshape(CT, P, KH, P).transpose(0, 3, 2, 1).reshape(CT, P, KH * P)
    ).astype(bf16)
    w_out2 = np.ascontiguousarray(
        W_out.reshape(MT, P, CT, P).transpose(0, 3, 2, 1).reshape(MT, P, CT * P)
    ).astype(bf16)
    wv = conv_w[:, 0, :]  # [C, 4]
    convw_all = np.ascontiguousarray(
        wv.reshape(CT, P, 4).transpose(1, 0, 2).reshape(P, CT * 4)
    ).astype(f32)
    biasf_all = np.ascontiguousarray(
        (conv_b + b_in * wv.sum(1)).reshape(CT, P).T
    ).astype(f32)
    bout2 = np.ascontiguousarray(b_out.reshape(MT, P).T).astype(f32)

    in_maps = []
    for core in range(NCORES):
        b, half = divmod(core, 2)
        hs = hidden_states[b, half * N:(half + 1) * N, :]
        hsT_arr = np.ascontiguousarray(hs.T).astype(bf16)
        if half == 0:
            xraw = np.repeat(-b_in[:, None], 3, axis=1)
        else:
            hs3 = hidden_states[b, half * N - 3:half * N, :]  # [3, H]
            xraw = W_in @ hs3.T  # [C, 3]
        halo_all = np.ascontiguousarray(
            xraw.reshape(CT, P, 3).transpose(1, 0, 2).reshape(P, CT * 3)
        ).astype(f32)
        in_maps.append(
            {
                "hsT": hsT_arr,
                "w_in": w_in2,
                "w_out": w_out2,
                "convw": convw_all,
                "biasf": biasf_all,
                "halo": halo_all,
                "bout": bout2,
            }
        )
    return in_maps


def kernel(hidden_states, W_in, b_in, conv_w, conv_b, W_out, b_out, trace=False):
    global _NC, LAST_RESULT
    from concourse.bass_utils import run_bass_kernel_spmd

    if _NC is None:
        _NC = _build_nc()
    in_maps = _prep_inputs(
        hidden_states, W_in, b_in, conv_w, conv_b, W_out, b_out
    )
    res = run_bass_kernel_spmd(_NC, in_maps, list(range(NCORES)), trace=trace)
    LAST_RESULT = res
    out = np.empty((B, S, H), np.float32)
    for core in range(NCORES):
        b, half = divmod(core, 2)
        out[b, half * N:(half + 1) * N, :] = res.results[core]["outT"].T
    return out


# revision 10
# speedup vs baseline: 5.8829x; 5.8829x over previous
"""Causal depthwise-conv MLP block (W_in -> causal conv K=4 -> SiLU -> W_out)
as a Bass/Tile kernel running data-parallel on 8 Trainium2 NeuronCores.

Sharding: (batch=4) x (sequence halves=2) -> 8 shards of 2048 sequence rows.
The causal conv needs 3 columns of left context; those are computed on the
host (exact fp32) and passed per-core, with the input-projection bias b_in
folded out of the conv input and into the SiLU bias so zero-padding at batch
starts is exact.

On-chip layout is channel-major ([C,seq] on partitions) so the depthwise conv
is per-partition scalar multiplies along the free dim.
"""

import numpy as np
import ml_dtypes

P = 128
B, S, H, C, K = 4, 4096, 2048, 4096, 4
NCORES = 8
N = S // 2          # sequence rows per core
KH = H // P         # 16 contraction tiles for the input projection
CT = C // P         # 32 channel tiles
MT = H // P         # 16 output row tiles
SUP = 1024          # sequence super-chunk held in SBUF as Y
NSUP = N // SUP     # 2
SUB = 512           # matmul moving free dim / PSUM bank
NSUB = SUP // SUB   # 2

_NC = None
LAST_RESULT = None


DEFAULT_BUFS = dict(hs=2, wi=2, wo=2, xs=2, ya=2, tm=1, ob=2, psA=2, psB=2)


def _build_nc(bufs=None):
    import concourse.bass as bass
    import concourse.bacc as bacc
    import concourse.mybir as mybir
    from concourse.tile import TileContext
    from contextlib import ExitStack

    nb = dict(DEFAULT_BUFS)
    if bufs:
        nb.update(bufs)

    fp32 = mybir.dt.float32
    bf16 = mybir.dt.bfloat16
    AF = mybir.ActivationFunctionType

    nc = bacc.Bacc()
    hsT = nc.declare_dram_parameter("hsT", [H, N], bf16, isOutput=False)
    w_in = nc.declare_dram_parameter("w_in", [CT, P, KH * P], bf16, isOutput=False)
    w_out = nc.declare_dram_parameter("w_out", [MT, P, CT * P], bf16, isOutput=False)
    convw = nc.declare_dram_parameter("convw", [P, CT * 4], fp32, isOutput=False)
    biasf = nc.declare_dram_parameter("biasf", [P, CT], fp32, isOutput=False)
    halo = nc.declare_dram_parameter("halo", [P, CT * 3], fp32, isOutput=False)
    bout = nc.declare_dram_parameter("bout", [P, MT], fp32, isOutput=False)
    outT = nc.declare_dram_parameter("outT", [H, N], fp32, isOutput=True)

    with TileContext(nc) as tc, ExitStack() as ctx:
        const = ctx.enter_context(tc.tile_pool(name="const", bufs=1))
        hs_pool = ctx.enter_context(tc.tile_pool(name="hs", bufs=nb["hs"]))
        wi_pool = ctx.enter_context(tc.tile_pool(name="wi", bufs=nb["wi"]))
        wo_pool = ctx.enter_context(tc.tile_pool(name="wo", bufs=nb["wo"]))
        xs_pool = ctx.enter_context(tc.tile_pool(name="xs", bufs=nb["xs"]))
        ya_pool = ctx.enter_context(tc.tile_pool(name="ya", bufs=nb["ya"]))
        tm_pool = ctx.enter_context(tc.tile_pool(name="tm", bufs=nb["tm"]))
        yb_pool = ctx.enter_context(tc.tile_pool(name="yb", bufs=1))
        ob_pool = ctx.enter_context(tc.tile_pool(name="ob", bufs=nb["ob"]))
        psA = ctx.enter_context(tc.tile_pool(name="psA", bufs=nb["psA"], space="PSUM"))
        psB = ctx.enter_context(tc.tile_pool(name="psB", bufs=nb["psB"], space="PSUM"))

        cw = const.tile([P, CT * 4], fp32, tag="cw")
        nc.sync.dma_start(out=cw, in_=convw[:, :])
        bf = const.tile([P, CT], fp32, tag="bf")
        nc.sync.dma_start(out=bf, in_=biasf[:, :])
        hl = const.tile([P, CT * 3], fp32, tag="hl")
        nc.sync.dma_start(out=hl, in_=halo[:, :])
        bo = const.tile([P, MT], fp32, tag="bo")
        nc.sync.dma_start(out=bo, in_=bout[:, :])
        # last 3 conv-input columns of each channel tile, carried across supers
        xtail = const.tile([P, CT * 3], fp32, tag="xtail")

        for s in range(NSUP):
            hst = hs_pool.tile([P, KH * SUP], bf16, tag="hs")
            for k in range(KH):
                nc.sync.dma_start(
                    out=hst[:, k * SUP:(k + 1) * SUP],
                    in_=hsT[k * P:(k + 1) * P, s * SUP:(s + 1) * SUP],
                )
            ybig = yb_pool.tile([P, CT * SUP], bf16, tag="yb")

            # Phase A: x = W_in @ hs (bf16 matmul, fp32 psum) -> conv -> silu -> Y
            for ci in range(CT):
                wi = wi_pool.tile([P, KH * P], bf16, tag="wi")
                nc.sync.dma_start(out=wi, in_=w_in[ci])
                xs = xs_pool.tile([P, 3 + SUP], fp32, tag="xs")
                if s == 0:
                    nc.vector.tensor_copy(xs[:, 0:3], hl[:, ci * 3:ci * 3 + 3])
                else:
                    nc.vector.tensor_copy(xs[:, 0:3], xtail[:, ci * 3:ci * 3 + 3])
                for sub in range(NSUB):
                    off = sub * SUB
                    px = psA.tile([P, SUB], fp32, tag="px")
                    for k in range(KH):
                        nc.tensor.matmul(
                            px,
                            wi[:, k * P:(k + 1) * P],
                            hst[:, k * SUP + off:k * SUP + off + SUB],
                            start=(k == 0),
                            stop=(k == KH - 1),
                        )
                    nc.scalar.copy(xs[:, 3 + off:3 + off + SUB], px)
                if s + 1 < NSUP:
                    nc.vector.tensor_copy(
                        xtail[:, ci * 3:ci * 3 + 3], xs[:, SUP:SUP + 3]
                    )
                ya = ya_pool.tile([P, SUP], fp32, tag="ya")
                nc.vector.tensor_scalar_mul(
                    ya, xs[:, 0:SUP], cw[:, ci * 4:ci * 4 + 1]
                )
                for t in range(1, 4):
                    tm = tm_pool.tile([P, SUP], fp32, tag="tm")
                    nc.vector.tensor_scalar_mul(
                        tm, xs[:, t:t + SUP], cw[:, ci * 4 + t:ci * 4 + t + 1]
                    )
                    nc.vector.tensor_add(ya, ya, tm)
                nc.scalar.activation(
                    ybig[:, ci * SUP:(ci + 1) * SUP],
                    ya,
                    AF.Silu,
                    bias=bf[:, ci:ci + 1],
                    scale=1.0,
                )

            # Phase B: out = W_out @ Y (accumulate over all channel tiles)
            for m in range(MT):
                wo = wo_pool.tile([P, CT * P], bf16, tag="wo")
                nc.sync.dma_start(out=wo, in_=w_out[m])
                for sub in range(NSUB):
                    off = sub * SUB
                    po = psB.tile([P, SUB], fp32, tag="po")
                    for ci2 in range(CT):
                        nc.tensor.matmul(
                            po,
                            wo[:, ci2 * P:(ci2 + 1) * P],
                            ybig[:, ci2 * SUP + off:ci2 * SUP + off + SUB],
                            start=(ci2 == 0),
                            stop=(ci2 == CT - 1),
                        )
                    ob = ob_pool.tile([P, SUB], fp32, tag="ob")
                    nc.scalar.activation(
                        ob, po, AF.Identity, bias=bo[:, m:m + 1], scale=1.0
                    )
                    nc.sync.dma_start(
                        out=outT[m * P:(m + 1) * P, s * SUP + off:s * SUP + off + SUB],
                        in_=ob,
                    )
    nc.finalize()
    return nc


def _prep_inputs(hidden_states, W_in, b_in, conv_w, conv_b, W_out, b_out):
    bf16 = ml_dtypes.bfloat16
    f32 = np.float32
    hidden_states = np.asarray(hidden_states, f32)
    W_in = np.asarray(W_in, f32)
    b_in = np.asarray(b_in, f32)
    conv_w = np.asarray(conv_w, f32)
    conv_b = np.asarray(conv_b, f32)
    W_out = np.asarray(W_out, f32)
    b_out = np.asarray(b_out, f32)

    w_in2 = np.ascontiguousarray(
        W_in.reshape(CT, P, KH, P).transpose(0, 3, 2, 1).reshape(CT, P, KH * P)
    ).astype(bf16)
    w_out2 = np.ascontiguousarray(
        W_out.reshape(MT, P, CT, P).transpose(0, 3, 2, 1).reshape(MT, P, CT * P)
    ).astype(bf16)
    wv = conv_w[:, 0, :]  # [C, 4]
    convw_all = np.ascontiguousarray(
        wv.reshape(CT, P, 4).transpose(1, 0, 2).reshape(P, CT * 4)
    ).astype(f32)
    biasf_all = np.ascontiguousarray(
        (conv_b + b_in * wv.sum(1)).reshape(CT, P).T
    ).astype(f32)
    bout2 = np.ascontiguousarray(b_out.reshape(MT, P).T).astype(f32)

    in_maps = []
    for core in range(NCORES):
        b, half = divmod(core, 2)
        hs = hidden_states[b, half * N:(half + 1) * N, :]
        hsT_arr = np.ascontiguousarray(hs.T).astype(bf16)
        if half == 0:
            xraw = np.repeat(-b_in[:, None], 3, axis=1)
        else:
            hs3 = hidden_states[b, half * N - 3:half * N, :]  # [3, H]
            xraw = W_in @ hs3.T  # [C, 3]
        halo_all = np.ascontiguousarray(
            xraw.reshape(CT, P, 3).transpose(1, 0, 2).reshape(P, CT * 3)
        ).astype(f32)
        in_maps.append(
            {
                "hsT": hsT_arr,
                "w_in": w_in2,
                "w_out": w_out2,
                "convw": convw_all,
                "biasf": biasf_all,
                "halo": halo_all,
                "bout": bout2,
            }
        )
    return in_maps


def kernel(hidden_states, W_in, b_in, conv_w, conv_b, W_out, b_out, trace=False):
    global _NC, LAST_RESULT
    from concourse.bass_utils import run_bass_kernel_spmd

    if _NC is None:
        _NC = _build_nc()
    in_maps = _prep_inputs(
        hidden_states, W_in, b_in, conv_w, conv_b, W_out, b_out
    )
    res = run_bass_kernel_spmd(_NC, in_maps, list(range(NCORES)), trace=trace)
    LAST_RESULT = res
    out = np.empty((B, S, H), np.float32)
    for core in range(NCORES):
        b, half = divmod(core, 2)
        out[b, half * N:(half + 1) * N, :] = res.results[core]["outT"].T
    return out
